# revision 22
# baseline (speedup 1.0000x reference)
"""Trainium2 Bass kernel for nn_Block_8744553415185 (sparse_attention).

kernel(**inputs) takes FULL inputs, returns FULL outputs
(out [4,1024,1024], q_attn [4,16,1024,1024], topk_attn [4,16,17,17]).

Sharding (8 cores): core = (b, g), b = batch 0..3, g = head-group 0..1
(8 heads each). QKV is tensor-parallel over the 3C output dim; cross-core
traffic is one 4KB pairwise AllReduce(max) (q_lin quantization row-scales
span all 3072 channels) plus a host-side sum of the two projection partials.

Device layout is feature-major [feature, seq] so contractions sit on SBUF
partitions. Quant path is exact-integer: x and W are quantized to ints
(|int|<=127, exact in bf16; round via the 1.5*2^23 magic constant =
round-to-nearest-even = jnp.round), the G = wint.T@xint matmul accumulates
exactly in fp32 PSUM (< 2^24), q_lin is reconstructed in f32
(sw[o]*sx[n]*G + qb[o]), re-quantized against the pair-AllReduced column
max, and q_attn = qint.T@kint is another exact integer matmul with the
row scale applied per-partition on evacuation and the column scale folded
into kint (bf16). Exact attention: S^T orientation, exp without max
subtraction (S bounded), softmax denominator via ones-augmented V-hat
[m, 65], divide on PSUM evacuation. q_attn uses S orientation (n on
partitions) so its 32MB/core DMA is contiguous.
"""

import numpy as np
import ml_dtypes
from contextlib import ExitStack

import concourse.bass as bass
import concourse.tile as tile
import concourse.bass_isa as bass_isa
from concourse import bacc, mybir
from concourse.bass import ts, ds
from concourse.bass_utils import run_bass_kernel_spmd
from concourse.masks import make_identity

F32 = mybir.dt.float32
BF16 = mybir.dt.bfloat16
AO = mybir.AluOpType
AF = mybir.ActivationFunctionType

B, N, C = 4, 1024, 1024
H, HD = 16, 64
NCORES = 8
MAGIC = float(np.float32(3 * 2.0 ** 22))  # 1.5*2^23 round-to-nearest-even
QMAX = 127.0
SCALE = HD ** -0.5  # 0.125
R1 = int(C * 0.25) + 1   # 257
R2 = int(HD * 0.25) + 1  # 17
CT = 8
GROUPS = [[0, 1], [2, 3], [4, 5], [6, 7]]

_CACHE = {}


def _build_nc():
    nc = bacc.Bacc("TRN2", debug=False, num_devices=NCORES)

    xT_d = nc.dram_tensor("xT", [C, N], F32, kind="ExternalInput")
    wqkT_d = nc.dram_tensor("wqkT", [C, 1024], BF16, kind="ExternalInput")
    wvT_d = nc.dram_tensor("wvT", [C, 512], BF16, kind="ExternalInput")
    wintT_d = nc.dram_tensor("wintT", [C, 1536], BF16, kind="ExternalInput")
    wtkT_d = nc.dram_tensor("wtkT", [R1, 1024], BF16, kind="ExternalInput")
    wpT_d = nc.dram_tensor("wpT", [512, 1024], BF16, kind="ExternalInput")
    qb_d = nc.dram_tensor("qb_p", [128, 12], F32, kind="ExternalInput")
    sw_d = nc.dram_tensor("sw_p", [128, 12], F32, kind="ExternalInput")
    bqk_d = nc.dram_tensor("bqk_p", [128, 8], F32, kind="ExternalInput")
    bv_d = nc.dram_tensor("bv_row", [1, 512], F32, kind="ExternalInput")
    bp_d = nc.dram_tensor("bp_p", [128, 8], F32, kind="ExternalInput")

    qattn_d = nc.dram_tensor("qattn_o", [8, N, N], F32, kind="ExternalOutput")
    y_d = nc.dram_tensor("y_o", [1024, N], F32, kind="ExternalOutput")
    tk_d = nc.dram_tensor("tk_o", [8, R2, R2], F32, kind="ExternalOutput")

    cc_in = nc.dram_tensor("cc_in", [1, N], F32)
    cc_out = nc.dram_tensor("cc_out", [1, N], F32)

    with tile.TileContext(nc) as tc, ExitStack() as ctx:
        def pool(name, bufs, space="SBUF"):
            return ctx.enter_context(tc.tile_pool(name=name, bufs=bufs, space=space))
        scratch_p = pool("scratch", 3)   # 12KB/p transient f32
        xbf_p = pool("xbf", 8)           # 16KB x bf16
        stg_p = pool("stg", 2)           # 8KB q_attn staging
        xint_p = pool("xint", 8)         # 16KB (later y staging)
        wcol_p = pool("wcol", 12)        # 12KB column-streamed weights
        wp_p = pool("wp", 4)             # 8KB
        qkbf_p = pool("qkbf", 8)         # 16KB
        aot_p = pool("aot", 4)           # 8KB
        qf_p = pool("qf", 8)             # 16KB
        qlin_p = pool("qlin", 8)         # 32KB f32
        pt_p = pool("pt", 3)             # 3KB
        vhat_p = pool("vhat", 8)         # ~8.3KB
        acc_p = pool("acc", 2)           # 8KB
        vec_p = pool("vec", 2)           # 8KB
        onerow_p = pool("onerow", 1)
        dnr_p = pool("dnr", 2)
        dn0_p = pool("dn0", 4)
        rb_p = pool("rb", 1)             # 2KB
        wtk_p = pool("wtk", 1)           # 6KB
        ttk_p = pool("ttk", 8)
        tts_p = pool("tts", 2)
        misc_p = pool("misc", 1)
        ps_p = pool("ps", 8, space="PSUM")
        if True:
            # =============== P0: load x, column scales, quantize ==========
            amax = acc_p.tile([128, N], F32, tag="acc")
            nc.vector.memset(amax[:], 0.0)
            xbf = []
            for ct in range(CT):
                xf = scratch_p.tile([128, N], F32, tag="scr")
                nc.sync.dma_start(xf[:], xT_d.ap()[ts(ct, 128), :])
                xb = xbf_p.tile([128, N], BF16, tag="xbf")
                nc.vector.tensor_copy(xb[:], xf[:])
                xa = scratch_p.tile([128, N], F32, tag="scr")
                nc.scalar.activation(xa[:], xf[:], AF.Abs)
                nc.vector.tensor_tensor(amax[:], amax[:], xa[:], AO.max)
                xbf.append(xb)
            colabs = vec_p.tile([128, N], F32, tag="vec")
            nc.gpsimd.partition_all_reduce(colabs[:], amax[:], 128,
                                           bass_isa.ReduceOp.max)
            sx = vec_p.tile([128, N], F32, tag="vec")
            nc.vector.tensor_scalar(sx[:], colabs[:], 1e-5, 1.0 / QMAX,
                                    AO.max, AO.mult)
            inv_sx = vec_p.tile([128, N], F32, tag="vec")
            nc.vector.reciprocal(inv_sx[:], sx[:])
            # re-stream xT in f32 and quantize (f32 precision for rounding)
            xint = []
            for ct in range(CT):
                xf2 = scratch_p.tile([128, N], F32, tag="scr")
                nc.sync.dma_start(xf2[:], xT_d.ap()[ts(ct, 128), :])
                tmp = scratch_p.tile([128, N], F32, tag="scr")
                nc.gpsimd.tensor_tensor(tmp[:], xf2[:], inv_sx[:], AO.mult)
                xi = xint_p.tile([128, N], BF16, tag="xint")
                nc.vector.tensor_scalar(xi[:], tmp[:], MAGIC, MAGIC,
                                        AO.add, AO.subtract)
                xint.append(xi)

            # =============== P1: exact QKV (qk), V-hat =====================
            bqk_t = misc_p.tile([128, 8], F32, tag="bqk")
            nc.sync.dma_start(bqk_t[:], bqk_d.ap())
            qkbf = []  # 0-3: qhat*0.125, 4-7: khat
            for wgrp in range(2):
                wq = []
                for ct in range(CT):
                    w = wcol_p.tile([128, 512], BF16, tag="wcol",
                                    name=f"wq{wgrp}_{ct}")
                    nc.sync.dma_start(w[:], wqkT_d.ap()[ts(ct, 128),
                                                        ts(wgrp, 512)])
                    wq.append(w)
                for oi in range(4):
                    ot = wgrp * 4 + oi
                    qk = qkbf_p.tile([128, N], BF16, tag="qkbf")
                    for ch in range(2):
                        pmm = ps_p.tile([128, 512], F32, tag="ps")
                        for ct in range(CT):
                            nc.tensor.matmul(pmm[:], wq[ct][:, ts(oi, 128)],
                                             xbf[ct][:, ts(ch, 512)],
                                             start=(ct == 0), stop=(ct == CT - 1))
                        if ot < 4:
                            nc.vector.tensor_scalar(qk[:, ts(ch, 512)], pmm[:],
                                                    bqk_t[:, ot:ot + 1], SCALE,
                                                    AO.add, AO.mult)
                        else:
                            nc.vector.tensor_scalar_add(qk[:, ts(ch, 512)],
                                                        pmm[:],
                                                        bqk_t[:, ot:ot + 1])
                    qkbf.append(qk)

            bvb = onerow_p.tile([128, 512], F32, tag="bvb")
            bv_row_t = onerow_p.tile([1, 512], F32, tag="bvr")
            nc.sync.dma_start(bv_row_t[:], bv_d.ap())
            nc.gpsimd.partition_broadcast(bvb[:], bv_row_t[:])
            vhat = [None] * 8
            for grp in range(2):
                pvs = [ps_p.tile([128, 512], F32, tag="ps", name=f"pv{grp}_{i}")
                       for i in range(4)]
                for ct in range(CT):
                    wv = wcol_p.tile([128, 512], BF16, tag="wcol",
                                     name=f"wv{grp}_{ct}")
                    nc.sync.dma_start(wv[:], wvT_d.ap()[ts(ct, 128), :])
                    for mi in range(4):
                        mt = grp * 4 + mi
                        nc.tensor.matmul(pvs[mi][:], xbf[ct][:, ts(mt, 128)],
                                         wv[:],
                                         start=(ct == 0), stop=(ct == CT - 1))
                for mi in range(4):
                    mt = grp * 4 + mi
                    vh = vhat_p.tile([128, 8, 65], BF16, tag="vhat")
                    nc.vector.tensor_tensor(
                        vh[:, :, 0:64],
                        pvs[mi][:].rearrange("p (h d) -> p h d", h=8),
                        bvb[:].rearrange("p (h d) -> p h d", h=8), AO.add)
                    nc.gpsimd.memset(vh[:, :, 64:65], 1.0)
                    vhat[mt] = vh

            # =============== topk path (tiny) ==============================
            wtk_tiles = []
            for i, (p0, sz) in enumerate([(0, 128), (128, 128), (256, 1)]):
                wt = wtk_p.tile([sz, 1024], BF16, tag=f"wtk{i}")
                nc.sync.dma_start(wt[:], wtkT_d.ap()[ds(p0, sz), :])
                wtk_tiles.append(wt)
            ttk = []
            for ot in range(8):
                ptk = ps_p.tile([128, R2], F32, tag="ps")
                nc.tensor.matmul(ptk[:], wtk_tiles[0][:, ts(ot, 128)],
                                 xbf[0][:, 0:R2], start=True, stop=False)
                nc.tensor.matmul(ptk[:], wtk_tiles[1][:, ts(ot, 128)],
                                 xbf[1][:, 0:R2], start=False, stop=False)
                nc.tensor.matmul(ptk[:], wtk_tiles[2][:, ts(ot, 128)],
                                 xbf[2][0:1, 0:R2], start=False, stop=True)
                tt = ttk_p.tile([128, R2], BF16, tag="ttk")
                if ot < 4:
                    nc.vector.tensor_scalar(tt[:], ptk[:], bqk_t[:, ot:ot + 1],
                                            SCALE, AO.add, AO.mult)
                else:
                    nc.vector.tensor_scalar_add(tt[:], ptk[:], bqk_t[:, ot:ot + 1])
                ttk.append(tt)
            for h in range(8):
                t, p = h // 2, h % 2
                ptt = ps_p.tile([R2, R2], F32, tag="ps")
                nc.tensor.matmul(ptt[:], ttk[t][ds(64 * p, 64), :],
                                 ttk[4 + t][ds(64 * p, 64), :],
                                 start=True, stop=True,
                                 tile_position=(64 * p, 0))
                tts = tts_p.tile([R2, R2], F32, tag="tts")
                nc.scalar.copy(tts[:], ptt[:])
                nc.sync.dma_start(tk_d.ap()[h], tts[:])

            # =============== P2+P4 interleaved ============================
            qb_t = misc_p.tile([128, 12], F32, tag="qb")
            nc.sync.dma_start(qb_t[:], qb_d.ap())
            sw_t = misc_p.tile([128, 12], F32, tag="sw")
            nc.sync.dma_start(sw_t[:], sw_d.ap())
            qmaxq = acc_p.tile([128, N], F32, tag="acc")
            nc.vector.memset(qmaxq[:], 0.0)
            qmaxv = onerow_p.tile([128, N], F32, tag="accv")
            nc.vector.memset(qmaxv[:], 0.0)
            qlin = []

            def g_group(wgrp):
                wf = []
                for ct in range(CT):
                    w = wcol_p.tile([128, 512], BF16, tag="wcol",
                                    name=f"wf{wgrp}_{ct}")
                    nc.sync.dma_start(w[:], wintT_d.ap()[ts(ct, 128),
                                                         ts(wgrp, 512)])
                    wf.append(w)
                for oi in range(4):
                    ot = wgrp * 4 + oi
                    dst = None
                    if ot < 8:
                        dst = qlin_p.tile([128, N], F32, tag="qlin",
                                          name=f"qlin{ot}")
                        qlin.append(dst)
                    for ch in range(2):
                        pg = ps_p.tile([128, 512], F32, tag="ps",
                                       name=f"pg{ot}_{ch}")
                        for ct in range(CT):
                            nc.tensor.matmul(pg[:], wf[ct][:, ts(oi, 128)],
                                             xint[ct][:, ts(ch, 512)],
                                             start=(ct == 0), stop=(ct == CT - 1))
                        if dst is not None:
                            piece = dst[:, ts(ch, 512)]
                        else:
                            pt_tmp = scratch_p.tile([128, 512], F32, tag="scr",
                                                    name=f"gv{ot}_{ch}")
                            piece = pt_tmp[:]
                        nc.vector.tensor_scalar_mul(piece, pg[:],
                                                    sw_t[:, ot:ot + 1])
                        nc.vector.tensor_tensor(piece, piece,
                                                sx[:, ts(ch, 512)], AO.mult)
                        nc.vector.tensor_scalar_add(piece, piece,
                                                    qb_t[:, ot:ot + 1])
                        if dst is None:
                            pabs = scratch_p.tile([128, 512], F32, tag="scr",
                                                  name=f"ga{ot}_{ch}")
                            nc.scalar.activation(pabs[:], piece, AF.Abs)
                            nc.vector.tensor_tensor(qmaxv[:, ts(ch, 512)],
                                                    qmaxv[:, ts(ch, 512)],
                                                    pabs[:], AO.max)

            aot = [aot_p.tile([128, N], BF16, tag="aot", name=f"aot{i}")
                   for i in range(4)]

            def p4_block(t):
                pso_all = {}
                for ch in range(2):
                    pso = [ps_p.tile([65, 512], F32, tag="ps",
                                     name=f"pso{t}_{ch}_{i}") for i in range(2)]
                    prev = None
                    for mt in range(8):
                        cur = []
                        for p in range(2):
                            pst = ps_p.tile([128, 512], F32, tag="ps",
                                            name=f"pst{t}_{ch}_{mt}_{p}")
                            nc.tensor.matmul(
                                pst[:],
                                qkbf[4 + t][ds(64 * p, 64), ts(mt, 128)],
                                qkbf[t][ds(64 * p, 64), ts(ch, 512)],
                                start=True, stop=True,
                                tile_position=(64 * p, 0))
                            ptile = pt_p.tile([128, 512], BF16, tag="pt")
                            nc.scalar.activation(ptile[:], pst[:], AF.Exp)
                            cur.append((mt, p, ptile))
                        if prev is not None:
                            for (pmt, pp, ptl) in prev:
                                nc.tensor.matmul(pso[pp][:],
                                                 vhat[pmt][:, 2 * t + pp, :],
                                                 ptl[:],
                                                 start=(pmt == 0), stop=False)
                        prev = cur
                    for (pmt, pp, ptl) in prev:
                        nc.tensor.matmul(pso[pp][:], vhat[pmt][:, 2 * t + pp, :],
                                         ptl[:], start=False, stop=True)
                    for p in range(2):
                        dnr = dnr_p.tile([65, 512], F32, tag="dnr",
                                         name=f"dnr{t}_{ch}_{p}")
                        nc.scalar.copy(dnr[64:65, :], pso[p][64:65, :])
                        dn0 = dn0_p.tile([1, 512], F32, tag="dn0",
                                         name=f"dn0_{t}_{ch}_{p}")
                        nc.sync.dma_start(dn0[:], dnr[64:65, :])
                        nc.vector.reciprocal(dn0[:], dn0[:])
                        pso_all[(ch, p)] = (pso[p], dn0)
                for ch in range(2):
                    for p in range(2):
                        psop, dn0 = pso_all[(ch, p)]
                        rbt = rb_p.tile([64, 512], F32, tag="rb")
                        nc.gpsimd.partition_broadcast(rbt[:], dn0[:])
                        nc.vector.tensor_tensor(
                            aot[t][ds(64 * p, 64), ts(ch, 512)],
                            psop[0:64, :], rbt[:], AO.mult)

            g_group(0)
            p4_block(0)
            g_group(1)
            p4_block(1)
            g_group(2)
            p4_block(2)

            # deferred column-max over stored qk q_lin tiles
            for ot in range(8):
                qa = scratch_p.tile([128, N], F32, tag="scr", name=f"qa{ot}")
                nc.scalar.activation(qa[:], qlin[ot][:], AF.Abs)
                nc.vector.tensor_tensor(qmaxq[:], qmaxq[:], qa[:], AO.max)
            nc.vector.tensor_tensor(qmaxq[:], qmaxq[:], qmaxv[:], AO.max)
            pmaxr = acc_p.tile([128, N], F32, tag="acc")
            nc.gpsimd.partition_all_reduce(pmaxr[:], qmaxq[:], 128,
                                           bass_isa.ReduceOp.max)
            nc.sync.dma_start(cc_in.ap(), pmaxr[0:1, :])
            nc.gpsimd.collective_compute(
                "AllReduce", AO.max, replica_groups=GROUPS,
                ins=[cc_in.ap()], outs=[cc_out.ap()])

            p4_block(3)

            gm_row = onerow_p.tile([1, N], F32, tag="gmr")
            nc.sync.dma_start(gm_row[:], cc_out.ap())
            gmax = vec_p.tile([128, N], F32, tag="vec")
            nc.gpsimd.partition_broadcast(gmax[:], gm_row[:])
            s2 = vec_p.tile([128, N], F32, tag="vec")
            nc.vector.tensor_scalar(s2[:], gmax[:], 1e-5, 1.0 / QMAX,
                                    AO.max, AO.mult)
            inv_s2 = vec_p.tile([128, N], F32, tag="vec")
            nc.vector.reciprocal(inv_s2[:], s2[:])
            mrow = onerow_p.tile([8, 128], F32, tag="mrow")
            nc.sync.dma_start(mrow[:], cc_out.ap().rearrange("o (a b) -> (o a) b", a=8))
            ident8 = onerow_p.tile([8, 8], F32, tag="id8")
            make_identity(nc, ident8[:])
            ptr = ps_p.tile([128, 8], F32, tag="ps")
            nc.tensor.transpose(ptr[:], mrow[:], ident8[:])
            s2q_p = onerow_p.tile([128, 8], F32, tag="s2qp")
            nc.vector.tensor_scalar(s2q_p[:], ptr[:], 1e-5, SCALE / QMAX,
                                    AO.max, AO.mult)

            qf = []  # 0-3: exact qint; 4-7: kint * s2[m] (bf16)
            for ot in range(8):
                tmp = scratch_p.tile([128, N], F32, tag="scr")
                nc.gpsimd.tensor_tensor(tmp[:], qlin[ot][:], inv_s2[:], AO.mult)
                qi = qf_p.tile([128, N], BF16, tag="qf")
                nc.vector.tensor_scalar(qi[:], tmp[:], MAGIC, MAGIC,
                                        AO.add, AO.subtract)
                if ot >= 4:
                    nc.gpsimd.tensor_tensor(qi[:], qi[:], s2[:], AO.mult)
                qf.append(qi)

            # =============== P3: quant S -> q_attn =========================
            for t in range(4):
                for nt in range(8):
                    for p in range(2):
                        h = 2 * t + p
                        stg = stg_p.tile([128, 1024], F32, tag="stg")
                        for ch in range(2):
                            psq = ps_p.tile([128, 512], F32, tag="ps")
                            nc.tensor.matmul(
                                psq[:], qf[t][ds(64 * p, 64), ts(nt, 128)],
                                qf[4 + t][ds(64 * p, 64), ts(ch, 512)],
                                start=True, stop=True,
                                tile_position=(64 * p, 0))
                            if (nt + ch + p) % 2 == 0:
                                nc.scalar.activation(stg[:, ts(ch, 512)],
                                                     psq[:], AF.Copy,
                                                     scale=s2q_p[:, nt:nt + 1])
                            else:
                                nc.vector.tensor_scalar_mul(
                                    stg[:, ts(ch, 512)], psq[:],
                                    s2q_p[:, nt:nt + 1])
                        nc.sync.dma_start(qattn_d.ap()[h, ts(nt, 128), :],
                                          stg[:])

            # =============== P5: projection partial ========================
            wp = []
            for ct4 in range(4):
                w = wp_p.tile([128, 1024], BF16, tag="wp")
                nc.sync.dma_start(w[:], wpT_d.ap()[ts(ct4, 128), :])
                wp.append(w)
            bp_t = misc_p.tile([128, 8], F32, tag="bp")
            nc.sync.dma_start(bp_t[:], bp_d.ap())
            for ot in range(8):
                for ch in range(2):
                    pj = ps_p.tile([128, 512], F32, tag="ps")
                    for ct4 in range(4):
                        nc.tensor.matmul(pj[:], wp[ct4][:, ts(ot, 128)],
                                         aot[ct4][:, ts(ch, 512)],
                                         start=(ct4 == 0), stop=(ct4 == 3))
                    ystg = xint_p.tile([128, 512], F32, tag="xint")
                    nc.vector.tensor_scalar_add(ystg[:], pj[:], bp_t[:, ot:ot + 1])
                    nc.sync.dma_start(y_d.ap()[ts(ot, 128), ts(ch, 512)], ystg[:])

    nc.finalize()
    return nc


def _scales_np(t):
    s = np.max(np.abs(t), axis=-1, keepdims=True)
    return np.maximum(s, np.float32(1e-5)) / np.float32(QMAX)


def _host_prep(x, W_qkv, b_qkv, W_proj, b_proj):
    x = np.ascontiguousarray(x, dtype=np.float32)
    W_qkv = np.ascontiguousarray(W_qkv, dtype=np.float32)
    b_qkv = np.ascontiguousarray(b_qkv, dtype=np.float32)
    W_proj = np.ascontiguousarray(W_proj, dtype=np.float32)
    b_proj = np.ascontiguousarray(b_proj, dtype=np.float32)
    bf = ml_dtypes.bfloat16

    sw = _scales_np(W_qkv)                       # [3C, 1]
    wint = np.clip(np.round(W_qkv / sw), -128, 127).astype(np.float32)
    sb = _scales_np(b_qkv[None, :])
    qb_vals = (np.clip(np.round(b_qkv[None, :] / sb), -128, 127) * sb)[0]

    in_maps = []
    for core in range(NCORES):
        b, g = divmod(core, 2)
        qr = slice(g * 512, g * 512 + 512)
        kr = slice(1024 + g * 512, 1024 + g * 512 + 512)
        vr = slice(2048 + g * 512, 2048 + g * 512 + 512)
        xT = np.ascontiguousarray(x[b].T)
        wqkT = np.ascontiguousarray(
            np.concatenate([W_qkv[qr], W_qkv[kr]], axis=0).T.astype(bf))
        wvT = np.ascontiguousarray(W_qkv[vr].T.astype(bf))
        wintT = np.ascontiguousarray(
            np.concatenate([wint[qr], wint[kr], wint[vr]], axis=0).T.astype(bf))
        wtkT = np.ascontiguousarray(
            np.concatenate([W_qkv[qr, :R1], W_qkv[kr, :R1]],
                           axis=0).T.astype(bf))
        wpT = np.ascontiguousarray(W_proj[:, g * 512:(g + 1) * 512].T.astype(bf))
        qb_core = np.concatenate([qb_vals[qr], qb_vals[kr], qb_vals[vr]])
        qb_p = np.ascontiguousarray(qb_core.reshape(12, 128).T)
        sw_core = np.concatenate([sw[qr, 0], sw[kr, 0], sw[vr, 0]])
        sw_p = np.ascontiguousarray(sw_core.reshape(12, 128).T)
        bqk = np.concatenate([b_qkv[qr], b_qkv[kr]])
        bqk_p = np.ascontiguousarray(bqk.reshape(8, 128).T)
        bv_row = np.ascontiguousarray(b_qkv[vr][None, :])
        bp = b_proj if g == 0 else np.zeros_like(b_proj)
        bp_p = np.ascontiguousarray(bp.reshape(8, 128).T)
        in_maps.append({
            "xT": xT, "wqkT": wqkT, "wvT": wvT, "wintT": wintT, "wtkT": wtkT,
            "wpT": wpT, "qb_p": qb_p, "sw_p": sw_p, "bqk_p": bqk_p,
            "bv_row": bv_row, "bp_p": bp_p,
        })
    return in_maps


def run(inputs, trace=False):
    if "nc" not in _CACHE:
        _CACHE["nc"] = _build_nc()
    nc = _CACHE["nc"]
    in_maps = _host_prep(**inputs)
    res = run_bass_kernel_spmd(nc, in_maps, list(range(NCORES)), trace=trace)

    out = np.empty((B, N, C), np.float32)
    q_attn = np.empty((B, H, N, N), np.float32)
    topk = np.empty((B, H, R2, R2), np.float32)
    for b in range(B):
        r0 = res.results[2 * b]
        r1 = res.results[2 * b + 1]
        out[b] = (r0["y_o"] + r1["y_o"]).T
        q_attn[b, 0:8] = r0["qattn_o"]
        q_attn[b, 8:16] = r1["qattn_o"]
        topk[b, 0:8] = r0["tk_o"]
        topk[b, 8:16] = r1["tk_o"]
    return (out, q_attn, topk), res


def kernel(x, W_qkv, b_qkv, W_proj, b_proj):
    (out, q_attn, topk), _ = run(
        dict(x=x, W_qkv=W_qkv, b_qkv=b_qkv, W_proj=W_proj, b_proj=b_proj))
    return out, q_attn, topk


# revision 23
# speedup vs baseline: 1.1099x; 1.1099x over previous
"""Trainium2 Bass kernel for nn_Block_8744553415185 (sparse_attention).

kernel(**inputs) takes FULL inputs, returns FULL outputs
(out [4,1024,1024], q_attn [4,16,1024,1024], topk_attn [4,16,17,17]).

Sharding (8 cores): core = (b, g), b = batch 0..3, g = head-group 0..1
(8 heads each). QKV is tensor-parallel over the 3C output dim; cross-core
traffic is one 4KB pairwise AllReduce(max) (q_lin quantization row-scales
span all 3072 channels) plus a host-side sum of the two projection partials.

Device layout is feature-major [feature, seq] so contractions sit on SBUF
partitions. Quant path is exact-integer: x and W are quantized to ints
(|int|<=127, exact in bf16; round via the 1.5*2^23 magic constant =
round-to-nearest-even = jnp.round), the G = wint.T@xint matmul accumulates
exactly in fp32 PSUM (< 2^24), q_lin is reconstructed in f32
(sw[o]*sx[n]*G + qb[o]), re-quantized against the pair-AllReduced column
max, and q_attn = qint.T@kint is another exact integer matmul with the
row scale applied per-partition on evacuation and the column scale folded
into kint (bf16). Exact attention: S^T orientation, exp without max
subtraction (S bounded), softmax denominator via ones-augmented V-hat
[m, 65], divide on PSUM evacuation. q_attn uses S orientation (n on
partitions) so its 32MB/core DMA is contiguous.
"""

import numpy as np
import ml_dtypes
from contextlib import ExitStack

import concourse.bass as bass
import concourse.tile as tile
import concourse.bass_isa as bass_isa
from concourse import bacc, mybir
from concourse.bass import ts, ds
from concourse.bass_utils import run_bass_kernel_spmd
from concourse.masks import make_identity

F32 = mybir.dt.float32
BF16 = mybir.dt.bfloat16
AO = mybir.AluOpType
AF = mybir.ActivationFunctionType

B, N, C = 4, 1024, 1024
H, HD = 16, 64
NCORES = 8
MAGIC = float(np.float32(3 * 2.0 ** 22))  # 1.5*2^23 round-to-nearest-even
QMAX = 127.0
SCALE = HD ** -0.5  # 0.125
R1 = int(C * 0.25) + 1   # 257
R2 = int(HD * 0.25) + 1  # 17
CT = 8
GROUPS = [[0, 1], [2, 3], [4, 5], [6, 7]]

_CACHE = {}


def _build_nc():
    nc = bacc.Bacc("TRN2", debug=False, num_devices=NCORES)

    xT_d = nc.dram_tensor("xT", [C, N], F32, kind="ExternalInput")
    wqkT_d = nc.dram_tensor("wqkT", [C, 1024], BF16, kind="ExternalInput")
    wvT_d = nc.dram_tensor("wvT", [C, 512], BF16, kind="ExternalInput")
    wintT_d = nc.dram_tensor("wintT", [C, 1536], BF16, kind="ExternalInput")
    wtkT_d = nc.dram_tensor("wtkT", [R1, 1024], BF16, kind="ExternalInput")
    wpT_d = nc.dram_tensor("wpT", [512, 1024], BF16, kind="ExternalInput")
    qb_d = nc.dram_tensor("qb_p", [128, 12], F32, kind="ExternalInput")
    sw_d = nc.dram_tensor("sw_p", [128, 12], F32, kind="ExternalInput")
    bqk_d = nc.dram_tensor("bqk_p", [128, 8], F32, kind="ExternalInput")
    bv_d = nc.dram_tensor("bv_row", [1, 512], F32, kind="ExternalInput")
    bp_d = nc.dram_tensor("bp_p", [128, 8], F32, kind="ExternalInput")

    qattn_d = nc.dram_tensor("qattn_o", [8, N, N], F32, kind="ExternalOutput")
    y_d = nc.dram_tensor("y_o", [1024, N], F32, kind="ExternalOutput")
    tk_d = nc.dram_tensor("tk_o", [8, R2, R2], F32, kind="ExternalOutput")

    cc_in = nc.dram_tensor("cc_in", [1, N], F32)
    cc_out = nc.dram_tensor("cc_out", [1, N], F32)

    with tile.TileContext(nc) as tc, ExitStack() as ctx:
        def pool(name, bufs, space="SBUF"):
            return ctx.enter_context(tc.tile_pool(name=name, bufs=bufs, space=space))
        scratch_p = pool("scratch", 3)   # 12KB/p transient f32
        xbf_p = pool("xbf", 8)           # 16KB x bf16
        xint_p = pool("xint", 8)         # 16KB (later y staging)
        wcol_p = pool("wcol", 12)        # 12KB column-streamed weights
        wp_p = pool("wp", 4)             # 8KB
        qkbf_p = pool("qkbf", 8)         # 16KB
        aot_p = pool("aot", 4)           # 8KB
        qf_p = pool("qf", 8)             # 16KB
        qlin_p = pool("qlin", 8)         # 32KB f32
        pt_p = pool("pt", 3)             # 3KB
        vhat_p = pool("vhat", 8)         # ~8.3KB
        acc_p = pool("acc", 2)           # 8KB
        vec_p = pool("vec", 2)           # 8KB
        onerow_p = pool("onerow", 1)
        dnr_p = pool("dnr", 2)
        dn0_p = pool("dn0", 4)
        rb_p = pool("rb", 1)             # 2KB
        wtk_p = pool("wtk", 1)           # 6KB
        ttk_p = pool("ttk", 8)
        tts_p = pool("tts", 2)
        misc_p = pool("misc", 1)
        ps_p = pool("ps", 8, space="PSUM")
        if True:
            # =============== P0: load x, column scales, quantize ==========
            amax = acc_p.tile([128, N], F32, tag="acc")
            nc.vector.memset(amax[:], 0.0)
            xbf = []
            for ct in range(CT):
                xf = scratch_p.tile([128, N], F32, tag="scr")
                nc.sync.dma_start(xf[:], xT_d.ap()[ts(ct, 128), :])
                xb = xbf_p.tile([128, N], BF16, tag="xbf")
                nc.vector.tensor_copy(xb[:], xf[:])
                xa = scratch_p.tile([128, N], F32, tag="scr")
                nc.scalar.activation(xa[:], xf[:], AF.Abs)
                nc.vector.tensor_tensor(amax[:], amax[:], xa[:], AO.max)
                xbf.append(xb)
            colabs = vec_p.tile([128, N], F32, tag="vec")
            nc.gpsimd.partition_all_reduce(colabs[:], amax[:], 128,
                                           bass_isa.ReduceOp.max)
            sx = vec_p.tile([128, N], F32, tag="vec")
            nc.vector.tensor_scalar(sx[:], colabs[:], 1e-5, 1.0 / QMAX,
                                    AO.max, AO.mult)
            inv_sx = vec_p.tile([128, N], F32, tag="vec")
            nc.vector.reciprocal(inv_sx[:], sx[:])
            # re-stream xT in f32 and quantize (f32 precision for rounding)
            xint = []
            for ct in range(CT):
                xf2 = scratch_p.tile([128, N], F32, tag="scr")
                nc.sync.dma_start(xf2[:], xT_d.ap()[ts(ct, 128), :])
                tmp = scratch_p.tile([128, N], F32, tag="scr")
                nc.gpsimd.tensor_tensor(tmp[:], xf2[:], inv_sx[:], AO.mult)
                xi = xint_p.tile([128, N], BF16, tag="xint")
                nc.vector.tensor_scalar(xi[:], tmp[:], MAGIC, MAGIC,
                                        AO.add, AO.subtract)
                xint.append(xi)

            # =============== P1: exact QKV (qk), V-hat =====================
            bqk_t = misc_p.tile([128, 8], F32, tag="bqk")
            nc.sync.dma_start(bqk_t[:], bqk_d.ap())
            qkbf = []  # 0-3: qhat*0.125, 4-7: khat
            for wgrp in range(2):
                wq = []
                for ct in range(CT):
                    w = wcol_p.tile([128, 512], BF16, tag="wcol",
                                    name=f"wq{wgrp}_{ct}")
                    nc.sync.dma_start(w[:], wqkT_d.ap()[ts(ct, 128),
                                                        ts(wgrp, 512)])
                    wq.append(w)
                for oi in range(4):
                    ot = wgrp * 4 + oi
                    qk = qkbf_p.tile([128, N], BF16, tag="qkbf")
                    for ch in range(2):
                        pmm = ps_p.tile([128, 512], F32, tag="ps")
                        for ct in range(CT):
                            nc.tensor.matmul(pmm[:], wq[ct][:, ts(oi, 128)],
                                             xbf[ct][:, ts(ch, 512)],
                                             start=(ct == 0), stop=(ct == CT - 1))
                        if ot < 4:
                            nc.vector.tensor_scalar(qk[:, ts(ch, 512)], pmm[:],
                                                    bqk_t[:, ot:ot + 1], SCALE,
                                                    AO.add, AO.mult)
                        else:
                            nc.vector.tensor_scalar_add(qk[:, ts(ch, 512)],
                                                        pmm[:],
                                                        bqk_t[:, ot:ot + 1])
                    qkbf.append(qk)

            bvb = onerow_p.tile([128, 512], F32, tag="bvb")
            bv_row_t = onerow_p.tile([1, 512], F32, tag="bvr")
            nc.sync.dma_start(bv_row_t[:], bv_d.ap())
            nc.gpsimd.partition_broadcast(bvb[:], bv_row_t[:])
            vhat = [None] * 8
            for grp in range(2):
                pvs = [ps_p.tile([128, 512], F32, tag="ps", name=f"pv{grp}_{i}")
                       for i in range(4)]
                for ct in range(CT):
                    wv = wcol_p.tile([128, 512], BF16, tag="wcol",
                                     name=f"wv{grp}_{ct}")
                    nc.sync.dma_start(wv[:], wvT_d.ap()[ts(ct, 128), :])
                    for mi in range(4):
                        mt = grp * 4 + mi
                        nc.tensor.matmul(pvs[mi][:], xbf[ct][:, ts(mt, 128)],
                                         wv[:],
                                         start=(ct == 0), stop=(ct == CT - 1))
                for mi in range(4):
                    mt = grp * 4 + mi
                    vh = vhat_p.tile([128, 8, 65], BF16, tag="vhat")
                    nc.vector.tensor_tensor(
                        vh[:, :, 0:64],
                        pvs[mi][:].rearrange("p (h d) -> p h d", h=8),
                        bvb[:].rearrange("p (h d) -> p h d", h=8), AO.add)
                    nc.gpsimd.memset(vh[:, :, 64:65], 1.0)
                    vhat[mt] = vh

            # =============== topk path (tiny) ==============================
            wtk_tiles = []
            for i, (p0, sz) in enumerate([(0, 128), (128, 128), (256, 1)]):
                wt = wtk_p.tile([sz, 1024], BF16, tag=f"wtk{i}")
                nc.sync.dma_start(wt[:], wtkT_d.ap()[ds(p0, sz), :])
                wtk_tiles.append(wt)
            ttk = []
            for ot in range(8):
                ptk = ps_p.tile([128, R2], F32, tag="ps")
                nc.tensor.matmul(ptk[:], wtk_tiles[0][:, ts(ot, 128)],
                                 xbf[0][:, 0:R2], start=True, stop=False)
                nc.tensor.matmul(ptk[:], wtk_tiles[1][:, ts(ot, 128)],
                                 xbf[1][:, 0:R2], start=False, stop=False)
                nc.tensor.matmul(ptk[:], wtk_tiles[2][:, ts(ot, 128)],
                                 xbf[2][0:1, 0:R2], start=False, stop=True)
                tt = ttk_p.tile([128, R2], BF16, tag="ttk")
                if ot < 4:
                    nc.vector.tensor_scalar(tt[:], ptk[:], bqk_t[:, ot:ot + 1],
                                            SCALE, AO.add, AO.mult)
                else:
                    nc.vector.tensor_scalar_add(tt[:], ptk[:], bqk_t[:, ot:ot + 1])
                ttk.append(tt)
            for h in range(8):
                t, p = h // 2, h % 2
                ptt = ps_p.tile([R2, R2], F32, tag="ps")
                nc.tensor.matmul(ptt[:], ttk[t][ds(64 * p, 64), :],
                                 ttk[4 + t][ds(64 * p, 64), :],
                                 start=True, stop=True,
                                 tile_position=(64 * p, 0))
                tts = tts_p.tile([R2, R2], F32, tag="tts")
                nc.scalar.copy(tts[:], ptt[:])
                nc.sync.dma_start(tk_d.ap()[h], tts[:])

            # =============== P2+P4 interleaved ============================
            qb_t = misc_p.tile([128, 12], F32, tag="qb")
            nc.sync.dma_start(qb_t[:], qb_d.ap())
            sw_t = misc_p.tile([128, 12], F32, tag="sw")
            nc.sync.dma_start(sw_t[:], sw_d.ap())
            qmaxq = acc_p.tile([128, N], F32, tag="acc")
            nc.vector.memset(qmaxq[:], 0.0)
            qmaxv = onerow_p.tile([128, N], F32, tag="accv")
            nc.vector.memset(qmaxv[:], 0.0)
            qlin = []

            def g_group(wgrp):
                wf = []
                for ct in range(CT):
                    w = wcol_p.tile([128, 512], BF16, tag="wcol",
                                    name=f"wf{wgrp}_{ct}")
                    nc.sync.dma_start(w[:], wintT_d.ap()[ts(ct, 128),
                                                         ts(wgrp, 512)])
                    wf.append(w)
                for oi in range(4):
                    ot = wgrp * 4 + oi
                    dst = None
                    if ot < 8:
                        dst = qlin_p.tile([128, N], F32, tag="qlin",
                                          name=f"qlin{ot}")
                        qlin.append(dst)
                    for ch in range(2):
                        pg = ps_p.tile([128, 512], F32, tag="ps",
                                       name=f"pg{ot}_{ch}")
                        for ct in range(CT):
                            nc.tensor.matmul(pg[:], wf[ct][:, ts(oi, 128)],
                                             xint[ct][:, ts(ch, 512)],
                                             start=(ct == 0), stop=(ct == CT - 1))
                        if dst is not None:
                            piece = dst[:, ts(ch, 512)]
                        else:
                            pt_tmp = scratch_p.tile([128, 512], F32, tag="scr",
                                                    name=f"gv{ot}_{ch}")
                            piece = pt_tmp[:]
                        nc.vector.tensor_scalar_mul(piece, pg[:],
                                                    sw_t[:, ot:ot + 1])
                        nc.vector.tensor_tensor(piece, piece,
                                                sx[:, ts(ch, 512)], AO.mult)
                        nc.vector.tensor_scalar_add(piece, piece,
                                                    qb_t[:, ot:ot + 1])
                        if dst is None:
                            pabs = scratch_p.tile([128, 512], F32, tag="scr",
                                                  name=f"ga{ot}_{ch}")
                            nc.scalar.activation(pabs[:], piece, AF.Abs)
                            nc.vector.tensor_tensor(qmaxv[:, ts(ch, 512)],
                                                    qmaxv[:, ts(ch, 512)],
                                                    pabs[:], AO.max)

            aot = [aot_p.tile([128, N], BF16, tag="aot", name=f"aot{i}")
                   for i in range(4)]

            def p4_block(t):
                pso_all = {}
                for ch in range(2):
                    pso = [ps_p.tile([65, 512], F32, tag="ps",
                                     name=f"pso{t}_{ch}_{i}") for i in range(2)]
                    prev = None
                    for mt in range(8):
                        cur = []
                        for p in range(2):
                            pst = ps_p.tile([128, 512], F32, tag="ps",
                                            name=f"pst{t}_{ch}_{mt}_{p}")
                            nc.tensor.matmul(
                                pst[:],
                                qkbf[4 + t][ds(64 * p, 64), ts(mt, 128)],
                                qkbf[t][ds(64 * p, 64), ts(ch, 512)],
                                start=True, stop=True,
                                tile_position=(64 * p, 0))
                            ptile = pt_p.tile([128, 512], BF16, tag="pt")
                            nc.scalar.activation(ptile[:], pst[:], AF.Exp)
                            cur.append((mt, p, ptile))
                        if prev is not None:
                            for (pmt, pp, ptl) in prev:
                                nc.tensor.matmul(pso[pp][:],
                                                 vhat[pmt][:, 2 * t + pp, :],
                                                 ptl[:],
                                                 start=(pmt == 0), stop=False)
                        prev = cur
                    for (pmt, pp, ptl) in prev:
                        nc.tensor.matmul(pso[pp][:], vhat[pmt][:, 2 * t + pp, :],
                                         ptl[:], start=False, stop=True)
                    for p in range(2):
                        dnr = dnr_p.tile([65, 512], F32, tag="dnr",
                                         name=f"dnr{t}_{ch}_{p}")
                        nc.scalar.copy(dnr[64:65, :], pso[p][64:65, :])
                        dn0 = dn0_p.tile([1, 512], F32, tag="dn0",
                                         name=f"dn0_{t}_{ch}_{p}")
                        nc.sync.dma_start(dn0[:], dnr[64:65, :])
                        nc.vector.reciprocal(dn0[:], dn0[:])
                        pso_all[(ch, p)] = (pso[p], dn0)
                for ch in range(2):
                    for p in range(2):
                        psop, dn0 = pso_all[(ch, p)]
                        rbt = rb_p.tile([64, 512], F32, tag="rb")
                        nc.gpsimd.partition_broadcast(rbt[:], dn0[:])
                        nc.vector.tensor_tensor(
                            aot[t][ds(64 * p, 64), ts(ch, 512)],
                            psop[0:64, :], rbt[:], AO.mult)

            g_group(0)
            p4_block(0)
            g_group(1)
            p4_block(1)
            g_group(2)

            # deferred column-max over stored qk q_lin tiles
            for ot in range(8):
                qa = scratch_p.tile([128, N], F32, tag="scr", name=f"qa{ot}")
                nc.scalar.activation(qa[:], qlin[ot][:], AF.Abs)
                nc.vector.tensor_tensor(qmaxq[:], qmaxq[:], qa[:], AO.max)
            nc.vector.tensor_tensor(qmaxq[:], qmaxq[:], qmaxv[:], AO.max)
            pmaxr = acc_p.tile([128, N], F32, tag="acc")
            nc.gpsimd.partition_all_reduce(pmaxr[:], qmaxq[:], 128,
                                           bass_isa.ReduceOp.max)
            nc.sync.dma_start(cc_in.ap(), pmaxr[0:1, :])
            nc.gpsimd.collective_compute(
                "AllReduce", AO.max, replica_groups=GROUPS,
                ins=[cc_in.ap()], outs=[cc_out.ap()])

            p4_block(2)
            p4_block(3)

            gm_row = onerow_p.tile([1, N], F32, tag="gmr")
            nc.sync.dma_start(gm_row[:], cc_out.ap())
            gmax = vec_p.tile([128, N], F32, tag="vec")
            nc.gpsimd.partition_broadcast(gmax[:], gm_row[:])
            s2 = vec_p.tile([128, N], F32, tag="vec")
            nc.vector.tensor_scalar(s2[:], gmax[:], 1e-5, 1.0 / QMAX,
                                    AO.max, AO.mult)
            inv_s2 = vec_p.tile([128, N], F32, tag="vec")
            nc.vector.reciprocal(inv_s2[:], s2[:])
            mrow = onerow_p.tile([8, 128], F32, tag="mrow")
            nc.sync.dma_start(mrow[:], cc_out.ap().rearrange("o (a b) -> (o a) b", a=8))
            ident8 = onerow_p.tile([8, 8], F32, tag="id8")
            make_identity(nc, ident8[:])
            ptr = ps_p.tile([128, 8], F32, tag="ps")
            nc.tensor.transpose(ptr[:], mrow[:], ident8[:])
            s2q_p = onerow_p.tile([128, 8], F32, tag="s2qp")
            nc.vector.tensor_scalar(s2q_p[:], ptr[:], 1e-5, SCALE / QMAX,
                                    AO.max, AO.mult)

            qf = []  # 0-3: exact qint; 4-7: kint * s2[m] (bf16)
            for ot in range(8):
                tmp = scratch_p.tile([128, N], F32, tag="scr")
                if ot < 4:
                    nc.gpsimd.tensor_tensor(tmp[:], qlin[ot][:], inv_s2[:], AO.mult)
                else:
                    nc.vector.tensor_tensor(tmp[:], qlin[ot][:], inv_s2[:], AO.mult)
                qi = qf_p.tile([128, N], BF16, tag="qf")
                nc.vector.tensor_scalar(qi[:], tmp[:], MAGIC, MAGIC,
                                        AO.add, AO.subtract)
                if ot >= 4:
                    nc.gpsimd.tensor_tensor(qi[:], qi[:], s2[:], AO.mult)
                qf.append(qi)

            # =============== P3: quant S -> q_attn =========================
            for t in range(4):
                for nt in range(8):
                    for p in range(2):
                        h = 2 * t + p
                        stg = qlin_p.tile([128, 1024], F32, tag="qlin")
                        for ch in range(2):
                            psq = ps_p.tile([128, 512], F32, tag="ps")
                            nc.tensor.matmul(
                                psq[:], qf[t][ds(64 * p, 64), ts(nt, 128)],
                                qf[4 + t][ds(64 * p, 64), ts(ch, 512)],
                                start=True, stop=True,
                                tile_position=(64 * p, 0))
                            if (nt + ch + p) % 2 == 0:
                                nc.scalar.activation(stg[:, ts(ch, 512)],
                                                     psq[:], AF.Copy,
                                                     scale=s2q_p[:, nt:nt + 1])
                            else:
                                nc.vector.tensor_scalar_mul(
                                    stg[:, ts(ch, 512)], psq[:],
                                    s2q_p[:, nt:nt + 1])
                        nc.sync.dma_start(qattn_d.ap()[h, ts(nt, 128), :],
                                          stg[:])

            # =============== P5: projection partial ========================
            wp = []
            for ct4 in range(4):
                w = wp_p.tile([128, 1024], BF16, tag="wp")
                nc.sync.dma_start(w[:], wpT_d.ap()[ts(ct4, 128), :])
                wp.append(w)
            bp_t = misc_p.tile([128, 8], F32, tag="bp")
            nc.sync.dma_start(bp_t[:], bp_d.ap())
            for ot in range(8):
                for ch in range(2):
                    pj = ps_p.tile([128, 512], F32, tag="ps")
                    for ct4 in range(4):
                        nc.tensor.matmul(pj[:], wp[ct4][:, ts(ot, 128)],
                                         aot[ct4][:, ts(ch, 512)],
                                         start=(ct4 == 0), stop=(ct4 == 3))
                    ystg = xint_p.tile([128, 512], F32, tag="xint")
                    nc.vector.tensor_scalar_add(ystg[:], pj[:], bp_t[:, ot:ot + 1])
                    nc.sync.dma_start(y_d.ap()[ts(ot, 128), ts(ch, 512)], ystg[:])

    nc.finalize()
    return nc


def _scales_np(t):
    s = np.max(np.abs(t), axis=-1, keepdims=True)
    return np.maximum(s, np.float32(1e-5)) / np.float32(QMAX)


def _host_prep(x, W_qkv, b_qkv, W_proj, b_proj):
    x = np.ascontiguousarray(x, dtype=np.float32)
    W_qkv = np.ascontiguousarray(W_qkv, dtype=np.float32)
    b_qkv = np.ascontiguousarray(b_qkv, dtype=np.float32)
    W_proj = np.ascontiguousarray(W_proj, dtype=np.float32)
    b_proj = np.ascontiguousarray(b_proj, dtype=np.float32)
    bf = ml_dtypes.bfloat16

    sw = _scales_np(W_qkv)                       # [3C, 1]
    wint = np.clip(np.round(W_qkv / sw), -128, 127).astype(np.float32)
    sb = _scales_np(b_qkv[None, :])
    qb_vals = (np.clip(np.round(b_qkv[None, :] / sb), -128, 127) * sb)[0]

    in_maps = []
    for core in range(NCORES):
        b, g = divmod(core, 2)
        qr = slice(g * 512, g * 512 + 512)
        kr = slice(1024 + g * 512, 1024 + g * 512 + 512)
        vr = slice(2048 + g * 512, 2048 + g * 512 + 512)
        xT = np.ascontiguousarray(x[b].T)
        wqkT = np.ascontiguousarray(
            np.concatenate([W_qkv[qr], W_qkv[kr]], axis=0).T.astype(bf))
        wvT = np.ascontiguousarray(W_qkv[vr].T.astype(bf))
        wintT = np.ascontiguousarray(
            np.concatenate([wint[qr], wint[kr], wint[vr]], axis=0).T.astype(bf))
        wtkT = np.ascontiguousarray(
            np.concatenate([W_qkv[qr, :R1], W_qkv[kr, :R1]],
                           axis=0).T.astype(bf))
        wpT = np.ascontiguousarray(W_proj[:, g * 512:(g + 1) * 512].T.astype(bf))
        qb_core = np.concatenate([qb_vals[qr], qb_vals[kr], qb_vals[vr]])
        qb_p = np.ascontiguousarray(qb_core.reshape(12, 128).T)
        sw_core = np.concatenate([sw[qr, 0], sw[kr, 0], sw[vr, 0]])
        sw_p = np.ascontiguousarray(sw_core.reshape(12, 128).T)
        bqk = np.concatenate([b_qkv[qr], b_qkv[kr]])
        bqk_p = np.ascontiguousarray(bqk.reshape(8, 128).T)
        bv_row = np.ascontiguousarray(b_qkv[vr][None, :])
        bp = b_proj if g == 0 else np.zeros_like(b_proj)
        bp_p = np.ascontiguousarray(bp.reshape(8, 128).T)
        in_maps.append({
            "xT": xT, "wqkT": wqkT, "wvT": wvT, "wintT": wintT, "wtkT": wtkT,
            "wpT": wpT, "qb_p": qb_p, "sw_p": sw_p, "bqk_p": bqk_p,
            "bv_row": bv_row, "bp_p": bp_p,
        })
    return in_maps


def run(inputs, trace=False):
    if "nc" not in _CACHE:
        _CACHE["nc"] = _build_nc()
    nc = _CACHE["nc"]
    in_maps = _host_prep(**inputs)
    res = run_bass_kernel_spmd(nc, in_maps, list(range(NCORES)), trace=trace)

    out = np.empty((B, N, C), np.float32)
    q_attn = np.empty((B, H, N, N), np.float32)
    topk = np.empty((B, H, R2, R2), np.float32)
    for b in range(B):
        r0 = res.results[2 * b]
        r1 = res.results[2 * b + 1]
        out[b] = (r0["y_o"] + r1["y_o"]).T
        q_attn[b, 0:8] = r0["qattn_o"]
        q_attn[b, 8:16] = r1["qattn_o"]
        topk[b, 0:8] = r0["tk_o"]
        topk[b, 8:16] = r1["tk_o"]
    return (out, q_attn, topk), res


def kernel(x, W_qkv, b_qkv, W_proj, b_proj):
    (out, q_attn, topk), _ = run(
        dict(x=x, W_qkv=W_qkv, b_qkv=b_qkv, W_proj=W_proj, b_proj=b_proj))
    return out, q_attn, topk
